# revision 6
# baseline (speedup 1.0000x reference)
"""MoE (top-2 of 8 experts, gated MLP) Trainium2 Bass kernel.

Strategy: D_MLP tensor-parallelism across the 8 NeuronCores. Every core
processes ALL routed (token, expert) pairs but only a 512-wide slice of
each expert's MLP hidden dimension, so compute is perfectly balanced and
each core reads exactly 1/8 of the expert weights (48 MB fp32).

Host side (cheap, <0.1% of FLOPs): router (softmax + top-2), dispatch
(gather tokens by expert, transposed layout), final combine (sum the 8
partial outputs, scatter-add the two pair rows of each token).

Device side (per core, all matmul FLOPs):
  for each expert e (segment of the pair list, bounds compiled in):
    preT[mc]  = Wg_e_slice[:, mc].T @ xT_seg     (PSUM, fp32r matmuls)
    preI[mc]  = Wi_e_slice[:, mc].T @ xT_seg
    hidT[mc]  = silu(preT[mc]) * preI[mc]        (ACT + DVE)
    y[mt]     = sum_mc hidT[mc][:, mt].T @ Wo_e_slice[mc]   (PSUM)
    y_sb      = y * w_pair[mt]                   (ACT copy w/ scale)
    DMA y_sb -> Y[segment rows]
All tensors stay in their natural layout; no transposes anywhere.
"""

import numpy as np

import concourse.bass as bass
import concourse.mybir as mybir
import concourse.tile as tile
from concourse import bacc
from concourse.bass_utils import run_bass_kernel_spmd

F32 = mybir.dt.float32
F32R = mybir.dt.float32r
AF = mybir.ActivationFunctionType

# Problem shape (hardcoded per contract)
T, D, DM, E, TOPK = 2048, 1024, 4096, 8, 2
NCORES = 8
SL = DM // NCORES  # 512: per-core slice of the MLP hidden dim
MC = SL // 128     # 4 mlp-slice chunks of 128
KC = D // 128      # 8 contraction chunks of 128
NTILE = 512        # pair-tile width for MM1/MM2 (fp32 moving max)


def _route(x, W_gate):
    """Replicates the reference router bit-for-bit in fp32 numpy."""
    h = np.asarray(x, np.float32).reshape(T, D)
    logits = h @ np.asarray(W_gate, np.float32)
    m = logits.max(-1, keepdims=True)
    p = np.exp(logits - m, dtype=np.float32)
    p /= p.sum(-1, keepdims=True)
    topi = np.argsort(-p, axis=-1, kind="stable")[:, :TOPK]
    topw = np.take_along_axis(p, topi, axis=-1)
    topw = topw / topw.sum(-1, keepdims=True)
    return h, topi, topw.astype(np.float32)


def _dispatch(h, topi, topw):
    """Group (token, expert) pairs by expert. Returns per-expert segment
    sizes, the gathered/transposed activations, pair weights and the
    token id of every pair row."""
    pair_tok, pair_w, segs = [], [], []
    for e in range(E):
        mask = topi == e  # [T, K]
        tok = np.nonzero(mask.any(-1))[0]
        kk = np.argmax(mask[tok], -1)
        pair_tok.append(tok)
        pair_w.append(topw[tok, kk])
        segs.append(len(tok))
    pair_tok = np.concatenate(pair_tok)
    pair_w = np.concatenate(pair_w).astype(np.float32)
    assert pair_tok.shape[0] == T * TOPK
    # gathered, transposed activations: [128, KC, P] with d = kc*128 + p0
    g = h[pair_tok]  # [P, D]
    xt = np.zeros((128, KC, T * TOPK + 8), np.float32)
    xt[:, :, :T * TOPK] = g.T.reshape(KC, 128, T * TOPK).transpose(1, 0, 2)
    return segs, xt, pair_w, pair_tok


def _mtiles(segs):
    """Global list of 128-row output tiles: (row0, m, tile_idx)."""
    tiles = []
    off = 0
    for ne in segs:
        r = 0
        while r < ne:
            m = min(128, ne - r)
            tiles.append((off + r, m))
            r += m
        off += ne
    return tiles


def build_program(segs, reps=1):
    """Builds the (SPMD, per-core) Bass program specialized to the
    per-expert segment sizes. Returns (nc, n_mtiles)."""
    mtiles = _mtiles(segs)
    nt = len(mtiles)

    nc = bacc.Bacc("TRN2", target_bir_lowering=False, debug=False,
                   num_devices=NCORES)
    P = T * TOPK
    xt_d = nc.dram_tensor("xt", [128, KC, P + 8], F32R, kind="ExternalInput")
    wg_d = nc.dram_tensor("wg", [E, 128, KC * MC * 128], F32R, kind="ExternalInput")
    wi_d = nc.dram_tensor("wi", [E, 128, KC * MC * 128], F32R, kind="ExternalInput")
    wo_d = nc.dram_tensor("wo", [E, 128, MC * D], F32R, kind="ExternalInput")
    wpt_d = nc.dram_tensor("wpt", [128, nt], F32, kind="ExternalInput")
    y_d = nc.dram_tensor("y", [P, D], F32, kind="ExternalOutput")

    with tile.TileContext(nc) as tc:
        with (
            tc.tile_pool(name="wpool", bufs=2) as wpool,
            tc.tile_pool(name="xpool", bufs=3) as xpool,
            tc.tile_pool(name="hpool", bufs=8) as hpool,
            tc.tile_pool(name="spool", bufs=2) as spool,
            tc.tile_pool(name="ypool", bufs=3) as ypool,
            tc.tile_pool(name="cpool", bufs=1) as cpool,
            tc.tile_pool(name="pgp", bufs=2, space="PSUM") as pgp,
            tc.tile_pool(name="pip", bufs=2, space="PSUM") as pip_,
            tc.tile_pool(name="pyp", bufs=2, space="PSUM") as pyp,
        ):
            wpt_sb = cpool.tile([128, nt], F32, name="wpt_sb")
            nc.sync.dma_start(wpt_sb[:, :], wpt_d[:, :])

            for _ in range(reps):
                gtile = 0
                off = 0
                for e in range(E):
                    ne = segs[e]
                    if ne == 0:
                        continue
                    wg_sb = wpool.tile([128, KC * MC * 128], F32R, tag="wg",
                                       name=f"wg{e}")
                    wi_sb = wpool.tile([128, KC * MC * 128], F32R, tag="wi",
                                       name=f"wi{e}")
                    wo_sb = wpool.tile([128, MC * D], F32R, tag="wo",
                                       name=f"wo{e}")
                    nc.sync.dma_start(wg_sb[:, :], wg_d[e, :, :])
                    nc.sync.dma_start(wi_sb[:, :], wi_d[e, :, :])
                    nc.sync.dma_start(wo_sb[:, :], wo_d[e, :, :])

                    j = 0
                    while j < ne:
                        ncols = min(NTILE, ne - j)
                        ncp = min(NTILE, (ncols + 3) // 4 * 4)
                        j0 = off + j
                        xt_sb = xpool.tile([128, KC, NTILE], F32R, tag="xt",
                                           name=f"xt{e}_{j}")
                        nc.sync.dma_start(xt_sb[:, :, :ncp],
                                          xt_d[:, :, j0:j0 + ncp])

                        hids = []
                        for mc in range(MC):
                            pg = pgp.tile([128, NTILE], F32, tag="pg", name=f"pg{e}_{j}_{mc}")
                            pi = pip_.tile([128, NTILE], F32, tag="pi", name=f"pi{e}_{j}_{mc}")
                            for k in range(KC):
                                nc.tensor.matmul(
                                    pg[:, :ncp],
                                    lhsT=wg_sb[:, (k * MC + mc) * 128:
                                               (k * MC + mc + 1) * 128],
                                    rhs=xt_sb[:, k, :ncp],
                                    start=(k == 0), stop=(k == KC - 1),
                                )
                            for k in range(KC):
                                nc.tensor.matmul(
                                    pi[:, :ncp],
                                    lhsT=wi_sb[:, (k * MC + mc) * 128:
                                               (k * MC + mc + 1) * 128],
                                    rhs=xt_sb[:, k, :ncp],
                                    start=(k == 0), stop=(k == KC - 1),
                                )
                            sg = spool.tile([128, NTILE], F32, tag="sg",
                                            name=f"sg{e}_{j}_{mc}")
                            nc.scalar.activation(sg[:, :ncp], pg[:, :ncp],
                                                 AF.Silu)
                            hid = hpool.tile([128, NTILE], F32R, tag="hid",
                                             name=f"hid{e}_{j}_{mc}")
                            nc.vector.tensor_mul(hid[:, :ncp], sg[:, :ncp],
                                                 pi[:, :ncp])
                            hids.append(hid)

                        r = 0
                        while r < ncols:
                            m = min(128, ncols - r)
                            mp = min(128, ncp - r)
                            y_sb = ypool.tile([128, D], F32, tag="ysb",
                                              name=f"y{e}_{j}_{r}")
                            for nh in range(2):
                                py = pyp.tile([128, 512], F32, tag="py",
                                              name=f"py{e}_{j}_{r}_{nh}")
                                for mc in range(MC):
                                    nc.tensor.matmul(
                                        py[:mp, :],
                                        lhsT=hids[mc][:, r:r + mp],
                                        rhs=wo_sb[:, mc * D + nh * 512:
                                                  mc * D + (nh + 1) * 512],
                                        start=(mc == 0), stop=(mc == MC - 1),
                                    )
                                nc.scalar.activation(
                                    y_sb[:m, nh * 512:(nh + 1) * 512],
                                    py[:m, :], AF.Copy,
                                    scale=wpt_sb[:m, gtile:gtile + 1],
                                )
                            nc.scalar.dma_start(y_d[j0 + r:j0 + r + m, :],
                                                y_sb[:m, :])
                            gtile += 1
                            r += m
                        j += ncols
                    off += ne
    nc.finalize()
    return nc, nt


def prepare_inputs(x, W_gate, We_gate, We_in, We_out):
    h, topi, topw = _route(x, W_gate)
    segs, xt, pair_w, pair_tok = _dispatch(h, topi, topw)
    mtiles = _mtiles(segs)
    nt = len(mtiles)
    wpt = np.zeros((128, nt), np.float32)
    for i, (r0, m) in enumerate(mtiles):
        wpt[:m, i] = pair_w[r0:r0 + m]

    Wg = np.asarray(We_gate, np.float32)
    Wi = np.asarray(We_in, np.float32)
    Wo = np.asarray(We_out, np.float32)
    in_maps = []
    for c in range(NCORES):
        sl = slice(c * SL, (c + 1) * SL)
        # [E, D, SL] -> [E, 128p, KC*MC*128] with d=k*128+p, col=(k*MC+mc)*128+m
        wg_c = np.ascontiguousarray(
            Wg[:, :, sl].reshape(E, KC, 128, MC, 128)
            .transpose(0, 2, 1, 3, 4).reshape(E, 128, KC * MC * 128))
        wi_c = np.ascontiguousarray(
            Wi[:, :, sl].reshape(E, KC, 128, MC, 128)
            .transpose(0, 2, 1, 3, 4).reshape(E, 128, KC * MC * 128))
        # [E, SL, D] -> [E, 128p, MC*D] with hid=mc*128+p
        wo_c = np.ascontiguousarray(
            Wo[:, sl, :].reshape(E, MC, 128, D)
            .transpose(0, 2, 1, 3).reshape(E, 128, MC * D))
        in_maps.append({"xt": xt, "wg": wg_c, "wi": wi_c, "wo": wo_c,
                        "wpt": wpt})
    return segs, in_maps, pair_tok


def combine(results, pair_tok, x_dtype):
    ysum = np.zeros((T * TOPK, D), np.float32)
    for r in results:
        ysum += r["y"]
    order = np.argsort(pair_tok, kind="stable")
    out = ysum[order[0::2]] + ysum[order[1::2]]
    return out.reshape(1, T, D).astype(x_dtype)


_cache = {}


def kernel(x, W_gate, We_gate, We_in, We_out):
    segs, in_maps, pair_tok = prepare_inputs(x, W_gate, We_gate, We_in, We_out)
    key = tuple(segs)
    if key not in _cache:
        _cache[key] = build_program(segs, reps=1)
    nc, _ = _cache[key]
    res = run_bass_kernel_spmd(nc, in_maps, list(range(NCORES)))
    return combine(res.results, pair_tok, np.asarray(x).dtype)


# revision 8
# speedup vs baseline: 38.5395x; 38.5395x over previous
"""MoE (top-2 of 8 experts, gated MLP) Trainium2 Bass kernel.

Strategy: D_MLP tensor-parallelism across the 8 NeuronCores. Every core
processes ALL routed (token, expert) pairs but only a 512-wide slice of
each expert's MLP hidden dimension, so compute is perfectly balanced and
each core reads exactly 1/8 of the expert weights (48 MB fp32).

Host side (cheap, <0.1% of FLOPs): router (softmax + top-2), dispatch
(gather tokens by expert, transposed layout), final combine (sum the 8
partial outputs, scatter-add the two pair rows of each token).

Device side (per core, all matmul FLOPs):
  for each expert e (segment of the pair list, bounds compiled in):
    preT[mc]  = Wg_e_slice[:, mc].T @ xT_seg     (PSUM, fp32r matmuls)
    preI[mc]  = Wi_e_slice[:, mc].T @ xT_seg
    hidT[mc]  = silu(preT[mc]) * preI[mc]        (ACT + DVE)
    y[mt]     = sum_mc hidT[mc][:, mt].T @ Wo_e_slice[mc]   (PSUM)
    y_sb      = y * w_pair[mt]                   (ACT copy w/ scale)
    DMA y_sb -> Y[segment rows]
All tensors stay in their natural layout; no transposes anywhere.
"""

import numpy as np

import concourse.bass as bass
import concourse.mybir as mybir
import concourse.tile as tile
from concourse import bacc
from concourse.bass_utils import run_bass_kernel_spmd

F32 = mybir.dt.float32
F32R = mybir.dt.float32r
AF = mybir.ActivationFunctionType

# Problem shape (hardcoded per contract)
T, D, DM, E, TOPK = 2048, 1024, 4096, 8, 2
NCORES = 8
SL = DM // NCORES  # 512: per-core slice of the MLP hidden dim
MC = SL // 128     # 4 mlp-slice chunks of 128
KC = D // 128      # 8 contraction chunks of 128
NTILE = 512        # pair-tile width for MM1/MM2 (fp32 moving max)


def _route(x, W_gate):
    """Replicates the reference router bit-for-bit in fp32 numpy."""
    h = np.asarray(x, np.float32).reshape(T, D)
    logits = h @ np.asarray(W_gate, np.float32)
    m = logits.max(-1, keepdims=True)
    p = np.exp(logits - m, dtype=np.float32)
    p /= p.sum(-1, keepdims=True)
    topi = np.argsort(-p, axis=-1, kind="stable")[:, :TOPK]
    topw = np.take_along_axis(p, topi, axis=-1)
    topw = topw / topw.sum(-1, keepdims=True)
    return h, topi, topw.astype(np.float32)


def _dispatch(h, topi, topw):
    """Group (token, expert) pairs by expert. Returns per-expert segment
    sizes, the gathered/transposed activations, pair weights and the
    token id of every pair row."""
    pair_tok, pair_w, segs = [], [], []
    for e in range(E):
        mask = topi == e  # [T, K]
        tok = np.nonzero(mask.any(-1))[0]
        kk = np.argmax(mask[tok], -1)
        pair_tok.append(tok)
        pair_w.append(topw[tok, kk])
        segs.append(len(tok))
    pair_tok = np.concatenate(pair_tok)
    pair_w = np.concatenate(pair_w).astype(np.float32)
    assert pair_tok.shape[0] == T * TOPK
    # gathered, transposed activations: [128, KC, P] with d = kc*128 + p0
    g = h[pair_tok]  # [P, D]
    xt = np.zeros((128, KC, T * TOPK + 8), np.float32)
    xt[:, :, :T * TOPK] = g.T.reshape(KC, 128, T * TOPK).transpose(1, 0, 2)
    return segs, xt, pair_w, pair_tok


def _mtiles(segs):
    """Global list of 128-row output tiles: (row0, m, tile_idx)."""
    tiles = []
    off = 0
    for ne in segs:
        r = 0
        while r < ne:
            m = min(128, ne - r)
            tiles.append((off + r, m))
            r += m
        off += ne
    return tiles


def build_program(segs, reps=1):
    """Builds the (SPMD, per-core) Bass program specialized to the
    per-expert segment sizes. Returns (nc, n_mtiles)."""
    mtiles = _mtiles(segs)
    nt = len(mtiles)

    nc = bacc.Bacc("TRN2", target_bir_lowering=False, debug=False,
                   num_devices=NCORES)
    P = T * TOPK
    xt_d = nc.dram_tensor("xt", [128, KC, P + 8], F32R, kind="ExternalInput")
    wg_d = nc.dram_tensor("wg", [E, 128, KC * MC * 128], F32R, kind="ExternalInput")
    wi_d = nc.dram_tensor("wi", [E, 128, KC * MC * 128], F32R, kind="ExternalInput")
    wo_d = nc.dram_tensor("wo", [E, 128, MC * D], F32R, kind="ExternalInput")
    wpt_d = nc.dram_tensor("wpt", [128, nt], F32, kind="ExternalInput")
    y_d = nc.dram_tensor("y", [P, D], F32, kind="ExternalOutput")

    with tile.TileContext(nc) as tc:
        with (
            tc.tile_pool(name="wpool", bufs=2) as wpool,
            tc.tile_pool(name="xpool", bufs=3) as xpool,
            tc.tile_pool(name="hpool", bufs=8) as hpool,
            tc.tile_pool(name="spool", bufs=2) as spool,
            tc.tile_pool(name="ypool", bufs=3) as ypool,
            tc.tile_pool(name="cpool", bufs=1) as cpool,
            tc.tile_pool(name="pgp", bufs=2, space="PSUM") as pgp,
            tc.tile_pool(name="pip", bufs=2, space="PSUM") as pip_,
            tc.tile_pool(name="pyp", bufs=2, space="PSUM") as pyp,
        ):
            wpt_sb = cpool.tile([128, nt], F32, name="wpt_sb")
            nc.sync.dma_start(wpt_sb[:, :], wpt_d[:, :])

            for _ in range(reps):
                gtile = 0
                off = 0
                for e in range(E):
                    ne = segs[e]
                    if ne == 0:
                        continue
                    wg_sb = wpool.tile([128, KC * MC * 128], F32R, tag="wg",
                                       name=f"wg{e}")
                    wi_sb = wpool.tile([128, KC * MC * 128], F32R, tag="wi",
                                       name=f"wi{e}")
                    wo_sb = wpool.tile([128, MC * D], F32R, tag="wo",
                                       name=f"wo{e}")
                    nc.sync.dma_start(wg_sb[:, :], wg_d[e, :, :])
                    nc.sync.dma_start(wi_sb[:, :], wi_d[e, :, :])
                    nc.sync.dma_start(wo_sb[:, :], wo_d[e, :, :])

                    j = 0
                    while j < ne:
                        ncols = min(NTILE, ne - j)
                        ncp = min(NTILE, (ncols + 3) // 4 * 4)
                        j0 = off + j
                        xt_sb = xpool.tile([128, KC, NTILE], F32R, tag="xt",
                                           name=f"xt{e}_{j}")
                        nc.sync.dma_start(xt_sb[:, :, :ncp],
                                          xt_d[:, :, j0:j0 + ncp])

                        hids = []
                        for mc in range(MC):
                            pg = pgp.tile([128, NTILE], F32, tag="pg", name=f"pg{e}_{j}_{mc}")
                            pi = pip_.tile([128, NTILE], F32, tag="pi", name=f"pi{e}_{j}_{mc}")
                            for k in range(KC):
                                nc.tensor.matmul(
                                    pg[:, :ncp],
                                    lhsT=wg_sb[:, (k * MC + mc) * 128:
                                               (k * MC + mc + 1) * 128],
                                    rhs=xt_sb[:, k, :ncp],
                                    start=(k == 0), stop=(k == KC - 1),
                                )
                            for k in range(KC):
                                nc.tensor.matmul(
                                    pi[:, :ncp],
                                    lhsT=wi_sb[:, (k * MC + mc) * 128:
                                               (k * MC + mc + 1) * 128],
                                    rhs=xt_sb[:, k, :ncp],
                                    start=(k == 0), stop=(k == KC - 1),
                                )
                            sg = spool.tile([128, NTILE], F32, tag="sg",
                                            name=f"sg{e}_{j}_{mc}")
                            nc.scalar.activation(sg[:, :ncp], pg[:, :ncp],
                                                 AF.Silu)
                            hid = hpool.tile([128, NTILE], F32R, tag="hid",
                                             name=f"hid{e}_{j}_{mc}")
                            nc.vector.tensor_mul(hid[:, :ncp], sg[:, :ncp],
                                                 pi[:, :ncp])
                            hids.append(hid)

                        r = 0
                        while r < ncols:
                            m = min(128, ncols - r)
                            mp = min(128, ncp - r)
                            y_sb = ypool.tile([128, D], F32, tag="ysb",
                                              name=f"y{e}_{j}_{r}")
                            for nh in range(2):
                                py = pyp.tile([128, 512], F32, tag="py",
                                              name=f"py{e}_{j}_{r}_{nh}")
                                for mc in range(MC):
                                    nc.tensor.matmul(
                                        py[:mp, :],
                                        lhsT=hids[mc][:, r:r + mp],
                                        rhs=wo_sb[:, mc * D + nh * 512:
                                                  mc * D + (nh + 1) * 512],
                                        start=(mc == 0), stop=(mc == MC - 1),
                                    )
                                nc.scalar.activation(
                                    y_sb[:m, nh * 512:(nh + 1) * 512],
                                    py[:m, :], AF.Copy,
                                    scale=wpt_sb[:m, gtile:gtile + 1],
                                )
                            nc.scalar.dma_start(y_d[j0 + r:j0 + r + m, :],
                                                y_sb[:m, :])
                            gtile += 1
                            r += m
                        j += ncols
                    off += ne
    nc.finalize()
    return nc, nt


def prepare_inputs(x, W_gate, We_gate, We_in, We_out):
    h, topi, topw = _route(x, W_gate)
    segs, xt, pair_w, pair_tok = _dispatch(h, topi, topw)
    mtiles = _mtiles(segs)
    nt = len(mtiles)
    wpt = np.zeros((128, nt), np.float32)
    for i, (r0, m) in enumerate(mtiles):
        wpt[:m, i] = pair_w[r0:r0 + m]

    Wg = np.asarray(We_gate, np.float32)
    Wi = np.asarray(We_in, np.float32)
    Wo = np.asarray(We_out, np.float32)
    in_maps = []
    for c in range(NCORES):
        sl = slice(c * SL, (c + 1) * SL)
        # [E, D, SL] -> [E, 128p, KC*MC*128] with d=k*128+p, col=(k*MC+mc)*128+m
        wg_c = np.ascontiguousarray(
            Wg[:, :, sl].reshape(E, KC, 128, MC, 128)
            .transpose(0, 2, 1, 3, 4).reshape(E, 128, KC * MC * 128))
        wi_c = np.ascontiguousarray(
            Wi[:, :, sl].reshape(E, KC, 128, MC, 128)
            .transpose(0, 2, 1, 3, 4).reshape(E, 128, KC * MC * 128))
        # [E, SL, D] -> [E, 128p, MC*D] with hid=mc*128+p
        wo_c = np.ascontiguousarray(
            Wo[:, sl, :].reshape(E, MC, 128, D)
            .transpose(0, 2, 1, 3).reshape(E, 128, MC * D))
        in_maps.append({"xt": xt, "wg": wg_c, "wi": wi_c, "wo": wo_c,
                        "wpt": wpt})
    return segs, in_maps, pair_tok


def combine(results, pair_tok, x_dtype):
    ysum = np.zeros((T * TOPK, D), np.float32)
    for r in results:
        ysum += r["y"]
    order = np.argsort(pair_tok, kind="stable")
    out = ysum[order[0::2]] + ysum[order[1::2]]
    return out.reshape(1, T, D).astype(x_dtype)


class Runner:
    """Compile-once executor for an SPMD Bass program on the 8 axon
    NeuronCores (same machinery as bass2jax.run_bass_via_pjrt, but the
    jitted executable and device-resident inputs persist across calls)."""

    def __init__(self, nc):
        import jax
        from jax.experimental.shard_map import shard_map
        from jax.sharding import Mesh, PartitionSpec
        from concourse import bass2jax

        bass2jax.install_neuronx_cc_hook()
        self.jax = jax
        self.nc = nc
        part_name = (nc.partition_id_tensor.name
                     if nc.partition_id_tensor else None)
        in_names, out_names, out_avals = [], [], []
        for alloc in nc.m.functions[0].allocations:
            if not isinstance(alloc, mybir.MemoryLocationSet):
                continue
            name = alloc.memorylocations[0].name
            if alloc.kind == "ExternalInput":
                if name != part_name:
                    in_names.append(name)
            elif alloc.kind == "ExternalOutput":
                out_names.append(name)
                out_avals.append(jax.core.ShapedArray(
                    tuple(alloc.tensor_shape), mybir.dt.np(alloc.dtype)))
        self.in_names = list(in_names)
        self.out_names = out_names
        self.out_avals = out_avals
        all_names = tuple(in_names + out_names
                          + ([part_name] if part_name else []))

        def _body(*args):
            operands = list(args)
            if part_name is not None:
                operands.append(bass2jax.partition_id_tensor())
            outs = bass2jax._bass_exec_p.bind(
                *operands,
                out_avals=tuple(out_avals),
                in_names=all_names,
                out_names=tuple(out_names),
                lowering_input_output_aliases=(),
                sim_require_finite=True,
                sim_require_nnan=True,
                nc=nc,
            )
            return tuple(outs)

        devices = jax.devices()[:NCORES]
        self.mesh = Mesh(np.asarray(devices), ("core",))
        n_args = len(in_names) + len(out_names)
        self.pspec = PartitionSpec("core")
        self.sharded = jax.jit(
            shard_map(_body, mesh=self.mesh,
                      in_specs=(self.pspec,) * n_args,
                      out_specs=(self.pspec,) * len(out_names),
                      check_rep=False),
            keep_unused=True,
        )

    def stage(self, in_maps):
        """device_put the per-core inputs (+ zeroed outputs) once."""
        from jax.sharding import NamedSharding
        sh = NamedSharding(self.mesh, self.pspec)
        args = []
        for name in self.in_names:
            cat = np.concatenate([np.asarray(m[name]) for m in in_maps], 0)
            args.append(self.jax.device_put(cat, sh))
        for av in self.out_avals:
            z = np.zeros((NCORES * av.shape[0], *av.shape[1:]), av.dtype)
            args.append(self.jax.device_put(z, sh))
        self.jax.block_until_ready(args)
        return args

    def run(self, staged):
        outs = self.sharded(*staged)
        self.jax.block_until_ready(outs)
        return outs

    def fetch(self, outs):
        """-> list (per core) of dict name -> np.ndarray"""
        res = []
        for c in range(NCORES):
            d = {}
            for i, name in enumerate(self.out_names):
                av = self.out_avals[i]
                d[name] = np.asarray(outs[i]).reshape(
                    NCORES, *av.shape)[c]
            res.append(d)
        return res


_cache = {}


def kernel(x, W_gate, We_gate, We_in, We_out):
    segs, in_maps, pair_tok = prepare_inputs(x, W_gate, We_gate, We_in, We_out)
    key = tuple(segs)
    if key not in _cache:
        nc, _ = build_program(segs, reps=1)
        _cache[key] = Runner(nc)
    runner = _cache[key]
    outs = runner.run(runner.stage(in_maps))
    return combine(runner.fetch(outs), pair_tok, np.asarray(x).dtype)


# revision 13
# speedup vs baseline: 957.1352x; 24.8352x over previous
"""MoE (top-2 of 8 experts, gated MLP) Trainium2 Bass kernel.

Strategy: D_MLP tensor-parallelism across the 8 NeuronCores. Every core
processes ALL routed (token, expert) pairs but only a 512-wide slice of
each expert's MLP hidden dimension, so compute is perfectly balanced and
each core reads exactly 1/8 of the expert weights (48 MB fp32).

Host side (cheap, <0.1% of FLOPs): router (softmax + top-2), dispatch
(gather tokens by expert, transposed layout), final combine (sum the 8
partial outputs, scatter-add the two pair rows of each token).

Device side (per core, all matmul FLOPs):
  for each expert e (segment of the pair list, bounds compiled in):
    preT[mc]  = Wg_e_slice[:, mc].T @ xT_seg     (PSUM, fp32r matmuls)
    preI[mc]  = Wi_e_slice[:, mc].T @ xT_seg
    hidT[mc]  = silu(preT[mc]) * preI[mc]        (ACT + DVE)
    y[mt]     = sum_mc hidT[mc][:, mt].T @ Wo_e_slice[mc]   (PSUM)
    y_sb      = y * w_pair[mt]                   (ACT copy w/ scale)
    DMA y_sb -> Y[segment rows]
All tensors stay in their natural layout; no transposes anywhere.
"""

import numpy as np

import concourse.bass as bass
import concourse.mybir as mybir
import concourse.tile as tile
from concourse import bacc
from concourse.bass_utils import run_bass_kernel_spmd

F32 = mybir.dt.float32
F32R = mybir.dt.float32r
AF = mybir.ActivationFunctionType

# Problem shape (hardcoded per contract)
T, D, DM, E, TOPK = 2048, 1024, 4096, 8, 2
NCORES = 8
SL = DM // NCORES  # 512: per-core slice of the MLP hidden dim
MC = SL // 128     # 4 mlp-slice chunks of 128
KC = D // 128      # 8 contraction chunks of 128
NTILE = 512        # pair-tile width for MM1/MM2 (fp32 moving max)


def _route(x, W_gate):
    """Replicates the reference router bit-for-bit in fp32 numpy."""
    h = np.asarray(x, np.float32).reshape(T, D)
    logits = h @ np.asarray(W_gate, np.float32)
    m = logits.max(-1, keepdims=True)
    p = np.exp(logits - m, dtype=np.float32)
    p /= p.sum(-1, keepdims=True)
    topi = np.argsort(-p, axis=-1, kind="stable")[:, :TOPK]
    topw = np.take_along_axis(p, topi, axis=-1)
    topw = topw / topw.sum(-1, keepdims=True)
    return h, topi, topw.astype(np.float32)


def _dispatch(h, topi, topw):
    """Group (token, expert) pairs by expert. Returns per-expert segment
    sizes, the gathered/transposed activations, pair weights and the
    token id of every pair row."""
    pair_tok, pair_w, segs = [], [], []
    for e in range(E):
        mask = topi == e  # [T, K]
        tok = np.nonzero(mask.any(-1))[0]
        kk = np.argmax(mask[tok], -1)
        pair_tok.append(tok)
        pair_w.append(topw[tok, kk])
        segs.append(len(tok))
    pair_tok = np.concatenate(pair_tok)
    pair_w = np.concatenate(pair_w).astype(np.float32)
    assert pair_tok.shape[0] == T * TOPK
    # gathered, transposed activations: [128, KC, P] with d = kc*128 + p0
    g = h[pair_tok]  # [P, D]
    xt = np.zeros((128, KC, T * TOPK + 8), np.float32)
    xt[:, :, :T * TOPK] = g.T.reshape(KC, 128, T * TOPK).transpose(1, 0, 2)
    return segs, xt, pair_w, pair_tok


def _mtiles(segs):
    """Global list of 128-row output tiles: (row0, m, tile_idx)."""
    tiles = []
    off = 0
    for ne in segs:
        r = 0
        while r < ne:
            m = min(128, ne - r)
            tiles.append((off + r, m))
            r += m
        off += ne
    return tiles


def build_program(segs, reps=1, y_bf16=True, w_bf16=False):
    """Builds the (SPMD, per-core) Bass program specialized to the
    per-expert segment sizes. Returns (nc, n_mtiles)."""
    BF16 = mybir.dt.bfloat16
    YDT = BF16 if y_bf16 else F32
    XDT = BF16 if w_bf16 else F32R    # moving operand of MM1/MM2
    WDT = BF16 if w_bf16 else F32R    # SBUF dtype of weights
    WDDT = F32 if w_bf16 else F32R    # DRAM dtype of weights (bits = fp32)
    HDT = BF16 if w_bf16 else F32R    # hid (lhsT of MM3)
    wdma = (lambda out, in_: nc.gpsimd.dma_start(out, in_)) if w_bf16 \
        else (lambda out, in_: nc.sync.dma_start(out, in_))
    mtiles = _mtiles(segs)
    nt = len(mtiles)

    nc = bacc.Bacc("TRN2", target_bir_lowering=False, debug=False,
                   num_devices=NCORES)
    P = T * TOPK
    xt_d = nc.dram_tensor("xt", [128, KC, P + 8], XDT, kind="ExternalInput")
    wg_d = nc.dram_tensor("wg", [E, 128, KC * MC * 128], WDDT, kind="ExternalInput")
    wi_d = nc.dram_tensor("wi", [E, 128, KC * MC * 128], WDDT, kind="ExternalInput")
    wo_d = nc.dram_tensor("wo", [E, 128, MC * D], WDDT, kind="ExternalInput")
    wpt_d = nc.dram_tensor("wpt", [128, nt], F32, kind="ExternalInput")
    y_d = nc.dram_tensor("y", [P, D], YDT, kind="ExternalOutput")

    with tile.TileContext(nc) as tc:
        with (
            tc.tile_pool(name="wpool", bufs=2) as wpool,
            tc.tile_pool(name="xpool", bufs=3) as xpool,
            tc.tile_pool(name="hpool", bufs=8) as hpool,
            tc.tile_pool(name="spool", bufs=2) as spool,
            tc.tile_pool(name="ypool", bufs=3) as ypool,
            tc.tile_pool(name="cpool", bufs=1) as cpool,
            tc.tile_pool(name="pgp", bufs=2, space="PSUM") as pgp,
            tc.tile_pool(name="pip", bufs=2, space="PSUM") as pip_,
            tc.tile_pool(name="pyp", bufs=2, space="PSUM") as pyp,
        ):
            wpt_sb = cpool.tile([128, nt], F32, name="wpt_sb")
            nc.sync.dma_start(wpt_sb[:, :], wpt_d[:, :])

            for _ in range(reps):
                gtile = 0
                off = 0
                for e in range(E):
                    ne = segs[e]
                    if ne == 0:
                        continue
                    wg_sb = wpool.tile([128, KC * MC * 128], WDT, tag="wg",
                                       name=f"wg{e}")
                    wi_sb = wpool.tile([128, KC * MC * 128], WDT, tag="wi",
                                       name=f"wi{e}")
                    wo_sb = wpool.tile([128, MC * D], WDT, tag="wo",
                                       name=f"wo{e}")
                    wdma(wg_sb[:, :], wg_d[e, :, :])
                    wdma(wi_sb[:, :], wi_d[e, :, :])
                    wdma(wo_sb[:, :], wo_d[e, :, :])

                    j = 0
                    while j < ne:
                        ncols = min(NTILE, ne - j)
                        ncp = min(NTILE, (ncols + 3) // 4 * 4)
                        j0 = off + j
                        xt_sb = xpool.tile([128, KC, NTILE], XDT, tag="xt",
                                           name=f"xt{e}_{j}")
                        nc.sync.dma_start(xt_sb[:, :, :ncp],
                                          xt_d[:, :, j0:j0 + ncp])

                        hids = []
                        for mc in range(MC):
                            pg = pgp.tile([128, NTILE], F32, tag="pg", name=f"pg{e}_{j}_{mc}")
                            pi = pip_.tile([128, NTILE], F32, tag="pi", name=f"pi{e}_{j}_{mc}")
                            for k in range(KC):
                                nc.tensor.matmul(
                                    pg[:, :ncp],
                                    lhsT=wg_sb[:, (k * MC + mc) * 128:
                                               (k * MC + mc + 1) * 128],
                                    rhs=xt_sb[:, k, :ncp],
                                    start=(k == 0), stop=(k == KC - 1),
                                )
                            for k in range(KC):
                                nc.tensor.matmul(
                                    pi[:, :ncp],
                                    lhsT=wi_sb[:, (k * MC + mc) * 128:
                                               (k * MC + mc + 1) * 128],
                                    rhs=xt_sb[:, k, :ncp],
                                    start=(k == 0), stop=(k == KC - 1),
                                )
                            sg = spool.tile([128, NTILE], F32, tag="sg",
                                            name=f"sg{e}_{j}_{mc}")
                            nc.scalar.activation(sg[:, :ncp], pg[:, :ncp],
                                                 AF.Silu)
                            hid = hpool.tile([128, NTILE], HDT, tag="hid",
                                             name=f"hid{e}_{j}_{mc}")
                            nc.vector.tensor_mul(hid[:, :ncp], sg[:, :ncp],
                                                 pi[:, :ncp])
                            hids.append(hid)

                        r = 0
                        while r < ncols:
                            m = min(128, ncols - r)
                            mp = min(128, ncp - r)
                            y_sb = ypool.tile([128, D], YDT, tag="ysb",
                                              name=f"y{e}_{j}_{r}")
                            for nh in range(2):
                                py = pyp.tile([128, 512], F32, tag="py",
                                              name=f"py{e}_{j}_{r}_{nh}")
                                for mc in range(MC):
                                    nc.tensor.matmul(
                                        py[:mp, :],
                                        lhsT=hids[mc][:, r:r + mp],
                                        rhs=wo_sb[:, mc * D + nh * 512:
                                                  mc * D + (nh + 1) * 512],
                                        start=(mc == 0), stop=(mc == MC - 1),
                                    )
                                nc.scalar.activation(
                                    y_sb[:m, nh * 512:(nh + 1) * 512],
                                    py[:m, :], AF.Copy,
                                    scale=wpt_sb[:m, gtile:gtile + 1],
                                )
                            nc.scalar.dma_start(y_d[j0 + r:j0 + r + m, :],
                                                y_sb[:m, :])
                            gtile += 1
                            r += m
                        j += ncols
                    off += ne
    nc.finalize()
    return nc, nt


def prepare_inputs(x, W_gate, We_gate, We_in, We_out, w_bf16=False):
    h, topi, topw = _route(x, W_gate)
    segs, xt, pair_w, pair_tok = _dispatch(h, topi, topw)
    if w_bf16:
        import ml_dtypes
        xt = xt.astype(ml_dtypes.bfloat16)
    mtiles = _mtiles(segs)
    nt = len(mtiles)
    wpt = np.zeros((128, nt), np.float32)
    for i, (r0, m) in enumerate(mtiles):
        wpt[:m, i] = pair_w[r0:r0 + m]

    Wg = np.asarray(We_gate, np.float32)
    Wi = np.asarray(We_in, np.float32)
    Wo = np.asarray(We_out, np.float32)
    in_maps = []
    for c in range(NCORES):
        sl = slice(c * SL, (c + 1) * SL)
        # [E, D, SL] -> [E, 128p, KC*MC*128] with d=k*128+p, col=(k*MC+mc)*128+m
        wg_c = np.ascontiguousarray(
            Wg[:, :, sl].reshape(E, KC, 128, MC, 128)
            .transpose(0, 2, 1, 3, 4).reshape(E, 128, KC * MC * 128))
        wi_c = np.ascontiguousarray(
            Wi[:, :, sl].reshape(E, KC, 128, MC, 128)
            .transpose(0, 2, 1, 3, 4).reshape(E, 128, KC * MC * 128))
        # [E, SL, D] -> [E, 128p, MC*D] with hid=mc*128+p
        wo_c = np.ascontiguousarray(
            Wo[:, sl, :].reshape(E, MC, 128, D)
            .transpose(0, 2, 1, 3).reshape(E, 128, MC * D))
        in_maps.append({"xt": xt, "wg": wg_c, "wi": wi_c, "wo": wo_c,
                        "wpt": wpt})
    return segs, in_maps, pair_tok


def combine(results, pair_tok, x_dtype):
    ysum = np.zeros((T * TOPK, D), np.float32)
    for r in results:
        ysum += np.asarray(r["y"], np.float32)
    order = np.argsort(pair_tok, kind="stable")
    out = ysum[order[0::2]] + ysum[order[1::2]]
    return out.reshape(1, T, D).astype(x_dtype)


class Runner:
    """Compile-once executor for an SPMD Bass program on the 8 axon
    NeuronCores (same machinery as bass2jax.run_bass_via_pjrt, but the
    jitted executable and device-resident inputs persist across calls)."""

    def __init__(self, nc):
        import jax
        from jax.experimental.shard_map import shard_map
        from jax.sharding import Mesh, PartitionSpec
        from concourse import bass2jax

        bass2jax.install_neuronx_cc_hook()
        self.jax = jax
        self.nc = nc
        part_name = (nc.partition_id_tensor.name
                     if nc.partition_id_tensor else None)
        in_names, out_names, out_avals = [], [], []
        for alloc in nc.m.functions[0].allocations:
            if not isinstance(alloc, mybir.MemoryLocationSet):
                continue
            name = alloc.memorylocations[0].name
            if alloc.kind == "ExternalInput":
                if name != part_name:
                    in_names.append(name)
            elif alloc.kind == "ExternalOutput":
                out_names.append(name)
                out_avals.append(jax.core.ShapedArray(
                    tuple(alloc.tensor_shape), mybir.dt.np(alloc.dtype)))
        self.in_names = list(in_names)
        self.out_names = out_names
        self.out_avals = out_avals
        all_names = tuple(in_names + out_names
                          + ([part_name] if part_name else []))

        def _body(*args):
            operands = list(args)
            if part_name is not None:
                operands.append(bass2jax.partition_id_tensor())
            outs = bass2jax._bass_exec_p.bind(
                *operands,
                out_avals=tuple(out_avals),
                in_names=all_names,
                out_names=tuple(out_names),
                lowering_input_output_aliases=(),
                sim_require_finite=True,
                sim_require_nnan=True,
                nc=nc,
            )
            return tuple(outs)

        devices = jax.devices()[:NCORES]
        self.mesh = Mesh(np.asarray(devices), ("core",))
        n_args = len(in_names) + len(out_names)
        self.pspec = PartitionSpec("core")
        self.sharded = jax.jit(
            shard_map(_body, mesh=self.mesh,
                      in_specs=(self.pspec,) * n_args,
                      out_specs=(self.pspec,) * len(out_names),
                      check_rep=False),
            keep_unused=True,
        )

    def stage(self, in_maps):
        """device_put the per-core inputs (+ zeroed outputs) once."""
        from jax.sharding import NamedSharding
        sh = NamedSharding(self.mesh, self.pspec)
        args = []
        for name in self.in_names:
            cat = np.concatenate([np.asarray(m[name]) for m in in_maps], 0)
            args.append(self.jax.device_put(cat, sh))
        for av in self.out_avals:
            z = np.zeros((NCORES * av.shape[0], *av.shape[1:]), av.dtype)
            args.append(self.jax.device_put(z, sh))
        self.jax.block_until_ready(args)
        return args

    def run(self, staged):
        outs = self.sharded(*staged)
        self.jax.block_until_ready(outs)
        return outs

    def fetch(self, outs):
        """-> list (per core) of dict name -> np.ndarray"""
        res = []
        for c in range(NCORES):
            d = {}
            for i, name in enumerate(self.out_names):
                av = self.out_avals[i]
                d[name] = np.asarray(outs[i]).reshape(
                    NCORES, *av.shape)[c]
            res.append(d)
        return res


_cache = {}


def kernel(x, W_gate, We_gate, We_in, We_out):
    segs, in_maps, pair_tok = prepare_inputs(x, W_gate, We_gate, We_in, We_out,
                                             w_bf16=True)
    key = tuple(segs)
    if key not in _cache:
        nc, _ = build_program(segs, reps=1, y_bf16=True, w_bf16=True)
        _cache[key] = Runner(nc)
    runner = _cache[key]
    outs = runner.run(runner.stage(in_maps))
    return combine(runner.fetch(outs), pair_tok, np.asarray(x).dtype)
